# revision 4
# baseline (speedup 1.0000x reference)
"""Trainium2 Bass kernel for a single-step attention decoder RNN.

Computation (batch=1, single step):
    x = relu(embedding[token])                    # [H]
    gates = x @ W_ih.T + b_ih + h0 @ W_hh.T + b_hh
    i, f, g, o = split(gates, 4)
    c = sig(f)*c0 + sig(i)*tanh(g); h = sig(o)*tanh(c)
    attn = enc @ h; ctx = attn @ enc
    ov = tanh(W_ctx @ [ctx; h] + b_ctx)
    logits = W_out @ ov + b_out; logp = log_softmax(logits)

Distribution over 8 NeuronCores:
  - LSTM gates H-sharded: core k computes the 128-wide slice k of each of
    i/f/g/o, hence its own h/c chunk; h chunks are AllGathered.
  - attention + W_ctx replicated (full h available after the AllGather).
  - W_out vocab-sharded (6400 rows/core, V padded 50257 -> 51200); padded
    rows get bias -30000 so they vanish under log_softmax.
  - log_softmax via per-core (max, sumexp) stats + a 16-float AllGather.

All matvecs run on the TensorEngine with the weight tile as the stationary
operand ([K=128, M=128] bf16, FWL) and a 1-column moving operand, so results
land partition-major (vocab/hidden on partitions) and the softmax tail is
fully lane-parallel.
"""

import os

import numpy as np
import ml_dtypes

import concourse.bacc as bacc
import concourse.bass as bass
import concourse.mybir as mybir
import concourse.tile as tile
from concourse.bass_utils import run_bass_kernel_spmd

F32 = mybir.dt.float32
BF16 = mybir.dt.bfloat16
NPBF16 = ml_dtypes.bfloat16
AF = mybir.ActivationFunctionType
AX = mybir.AxisListType

H = 1024
L = 40
V = 50257
NCORES = 8
VS = 6400          # vocab rows per core (padded)
VP = VS * NCORES   # 51200
NV = VS // 128     # 50 vocab tiles per core
KH = H // 128      # 8 k-tiles over H
K2H = 2 * H // 128  # 16 k-tiles over 2H
PAD_BIAS = -30000.0

_CACHE: dict = {}
LAST_RESULTS = None


def _build():
    """Build + compile the (SPMD) Bass program once."""
    nc = bacc.Bacc(
        "TRN2", target_bir_lowering=False, debug=False,
        enable_asserts=True, num_devices=NCORES,
    )

    # ---- I/O ----
    x_in = nc.dram_tensor("x_sw", [128, KH], F32, kind="ExternalInput")
    h0_in = nc.dram_tensor("h0_sw", [128, KH], F32, kind="ExternalInput")
    c0_in = nc.dram_tensor("c0_ch", [128, 1], F32, kind="ExternalInput")
    enc_in = nc.dram_tensor("enc16", [L, H], BF16, kind="ExternalInput")
    encT_in = nc.dram_tensor("encT16", [128, KH * L], BF16, kind="ExternalInput")
    wcat_in = nc.dram_tensor("wcat16", [128, 4 * K2H * 128], BF16, kind="ExternalInput")
    wctx_in = nc.dram_tensor("wctx16", [128, KH * K2H * 128], BF16, kind="ExternalInput")
    wout_in = nc.dram_tensor("wout16", [128, NV * KH * 128], BF16, kind="ExternalInput")
    bih_in = nc.dram_tensor("bih_sw", [128, 4], F32, kind="ExternalInput")
    bhh_in = nc.dram_tensor("bhh_sw", [128, 4], F32, kind="ExternalInput")
    bctx_in = nc.dram_tensor("bctx_sw", [128, KH], F32, kind="ExternalInput")
    bout_in = nc.dram_tensor("bout_sw", [128, NV], F32, kind="ExternalInput")
    ident_in = nc.dram_tensor("ident", [128, 128], F32, kind="ExternalInput")
    ones_in = nc.dram_tensor("ones1", [1, 128], F32, kind="ExternalInput")

    out_logp = nc.dram_tensor("out_logp", [128, NV], F32, kind="ExternalOutput")
    out_h = nc.dram_tensor("out_h", [128, 1], F32, kind="ExternalOutput")
    out_c = nc.dram_tensor("out_c", [128, 1], F32, kind="ExternalOutput")
    out_attn = nc.dram_tensor("out_attn", [L, 1], F32, kind="ExternalOutput")

    rg = [list(range(NCORES))]
    WCH = 2            # vocab tiles per wout DMA chunk
    NCHUNK = NV // WCH  # 25

    with tile.TileContext(nc) as tc:
        with (
            tc.tile_pool(name="wpool", bufs=1) as wpool,
            tc.tile_pool(name="work", bufs=1) as work,
            tc.tile_pool(name="psum", bufs=1, space="PSUM") as pp,
            tc.tile_pool(name="dram", bufs=1, space="DRAM") as dp,
        ):
            # ---- small loads (sync = HWDGE, keeps latency path clear) ----
            x_s = work.tile([128, KH], F32)
            h0_s = work.tile([128, KH], F32)
            c0_s = work.tile([128, 1], F32)
            enc_s = work.tile([L, H], BF16)
            encT_s = work.tile([128, KH * L], BF16)
            bih_s = work.tile([128, 4], F32)
            bhh_s = work.tile([128, 4], F32)
            bctx_s = work.tile([128, KH], F32)
            bout_s = work.tile([128, NV], F32)
            ident_s = work.tile([128, 128], F32)
            ones_s = work.tile([1, 128], F32)
            for dst, src in [
                (x_s, x_in), (h0_s, h0_in), (c0_s, c0_in), (enc_s, enc_in),
                (encT_s, encT_in), (bih_s, bih_in), (bhh_s, bhh_in),
                (bctx_s, bctx_in), (bout_s, bout_in), (ident_s, ident_in),
                (ones_s, ones_in),
            ]:
                nc.sync.dma_start(dst, src[:])

            # ---- bulk weight loads (gpsimd = SWDGE queue) ----
            wcat_s = wpool.tile([128, 4 * K2H * 128], BF16)
            nc.scalar.dma_start(wcat_s, wcat_in[:])
            wctx_s = wpool.tile([128, KH * K2H * 128], BF16)
            nc.scalar.dma_start(wctx_s, wctx_in[:])
            wv = []
            for ci in range(NCHUNK):
                t = wpool.tile([128, WCH * KH * 128], BF16, tag="wout",
                               bufs=NCHUNK, name=f"wv{ci}")
                nc.scalar.dma_start(t, wout_in[:, ci * WCH * KH * 128:(ci + 1) * WCH * KH * 128])
                wv.append(t)

            # ---- z = [relu(x); h0] in bf16 ----
            z16 = work.tile([128, K2H], BF16)
            nc.scalar.activation(z16[:, 0:KH], x_s, AF.Relu)
            nc.vector.tensor_copy(z16[:, KH:K2H], h0_s)

            # ---- gates: psum[:, m] = sum_k Wcat[m,k].T @ z[k] ----
            g_ps = pp.tile([128, 4], F32)
            for m in range(4):
                for k in range(K2H):
                    t0 = (m * K2H + k) * 128
                    nc.tensor.matmul(
                        g_ps[:, m:m + 1], wcat_s[:, t0:t0 + 128], z16[:, k:k + 1],
                        start=(k == 0), stop=(k == K2H - 1),
                    )

            gates = work.tile([128, 4], F32)
            nc.vector.tensor_add(gates, g_ps, bih_s)
            nc.vector.tensor_add(gates, gates, bhh_s)
            acts = work.tile([128, 4], F32)
            nc.scalar.activation(acts[:, 0:2], gates[:, 0:2], AF.Sigmoid)
            nc.scalar.activation(acts[:, 3:4], gates[:, 3:4], AF.Sigmoid)
            nc.scalar.activation(acts[:, 2:3], gates[:, 2:3], AF.Tanh)
            fc = work.tile([128, 1], F32)
            ig = work.tile([128, 1], F32)
            nc.vector.tensor_mul(fc, acts[:, 1:2], c0_s)
            nc.vector.tensor_mul(ig, acts[:, 0:1], acts[:, 2:3])
            c_new = work.tile([128, 1], F32)
            nc.vector.tensor_add(c_new, fc, ig)
            tanh_c = work.tile([128, 1], F32)
            nc.scalar.activation(tanh_c, c_new, AF.Tanh)
            h_new = work.tile([128, 1], F32)
            nc.vector.tensor_mul(h_new, acts[:, 3:4], tanh_c)
            nc.sync.dma_start(out_c[:], c_new)
            nc.sync.dma_start(out_h[:], h_new)

            # ---- AllGather h chunks -> full h [1024] ----
            ag1_in = dp.tile([128, 1], F32)
            ag1_out = dp.tile([H, 1], F32, name="ag1_out", addr_space="Shared")
            nc.sync.dma_start(ag1_in, h_new)
            nc.gpsimd.collective_compute(
                "AllGather", mybir.AluOpType.bypass, replica_groups=rg,
                ins=[ag1_in.opt()], outs=[ag1_out.opt()],
            )
            hf = work.tile([128, KH], F32)
            nc.sync.dma_start(hf, ag1_out.rearrange("(k p) one -> p (k one)", p=128))
            h16 = work.tile([128, KH], BF16)
            nc.vector.tensor_copy(h16, hf)

            # ---- attention: attn = enc @ h ; ctx = attn @ enc ----
            at_ps = pp.tile([L, 1], F32, tag="small", bufs=2)
            for k in range(KH):
                nc.tensor.matmul(
                    at_ps, encT_s[:, k * L:(k + 1) * L], h16[:, k:k + 1],
                    start=(k == 0), stop=(k == KH - 1),
                )
            attn_f = work.tile([L, 1], F32)
            nc.vector.tensor_copy(attn_f, at_ps)
            nc.sync.dma_start(out_attn[:], attn_f)
            a16 = work.tile([L, 1], BF16)
            nc.vector.tensor_copy(a16, at_ps)

            ctx_ps = pp.tile([128, KH], F32)
            for k in range(KH):
                nc.tensor.matmul(
                    ctx_ps[:, k:k + 1], enc_s[:, k * 128:(k + 1) * 128], a16,
                    start=True, stop=True,
                )

            z2 = work.tile([128, K2H], BF16)
            nc.vector.tensor_copy(z2[:, 0:KH], ctx_ps)
            nc.vector.tensor_copy(z2[:, KH:K2H], h16)

            # ---- out_vec = tanh(W_ctx @ [ctx; h] + b_ctx), full, replicated ----
            ov_ps = pp.tile([128, KH], F32)
            for m in range(KH):
                for k in range(K2H):
                    t0 = (m * K2H + k) * 128
                    nc.tensor.matmul(
                        ov_ps[:, m:m + 1], wctx_s[:, t0:t0 + 128], z2[:, k:k + 1],
                        start=(k == 0), stop=(k == K2H - 1),
                    )
            u = work.tile([128, KH], F32)
            nc.vector.tensor_add(u, ov_ps, bctx_s)
            ov16 = work.tile([128, KH], BF16)
            nc.scalar.activation(ov16, u, AF.Tanh)

            # ---- logits shard: psum[:, v] = sum_k Wout[v,k].T @ ov[k] ----
            lg_ps = pp.tile([128, NV], F32)
            for ci in range(NCHUNK):
                for vv in range(WCH):
                    v = ci * WCH + vv
                    for k in range(KH):
                        t0 = (vv * KH + k) * 128
                        nc.tensor.matmul(
                            lg_ps[:, v:v + 1], wv[ci][:, t0:t0 + 128], ov16[:, k:k + 1],
                            start=(k == 0), stop=(k == KH - 1),
                        )
            logits = work.tile([128, NV], F32)
            nc.vector.tensor_add(logits, lg_ps, bout_s)

            # ---- local softmax stats ----
            st2 = work.tile([128, 2], F32)
            nc.vector.reduce_max(st2[:, 0:1], logits, axis=AX.X)
            negm = work.tile([128, 1], F32)
            nc.vector.tensor_scalar_mul(negm, st2[:, 0:1], -1.0)
            nc.vector.memset(st2[:, 1:2], 0.0)
            etile = work.tile([128, NV], F32)
            nc.scalar.activation(etile, logits, AF.Exp, bias=negm, scale=1.0,
                                 accum_out=st2[:, 1:2])
            t1_ps = pp.tile([1, 128], F32, tag="small", bufs=2, name="t1_ps")
            nc.tensor.transpose(t1_ps, st2[:, 0:1], ident_s)
            t2_ps = pp.tile([1, 128], F32, tag="small", bufs=2, name="t2_ps")
            nc.tensor.transpose(t2_ps, st2[:, 1:2], ident_s)
            mloc = work.tile([1, 1], F32)
            nc.vector.reduce_max(mloc, t1_ps, axis=AX.X)
            negml = work.tile([1, 1], F32)
            nc.vector.tensor_scalar_mul(negml, mloc, -1.0)
            esh = work.tile([1, 128], F32)
            nc.scalar.activation(esh, t1_ps, AF.Exp, bias=negml, scale=1.0)
            swt = work.tile([1, 128], F32)
            nc.vector.tensor_mul(swt, esh, t2_ps)
            sloc = work.tile([1, 1], F32)
            nc.vector.reduce_sum(sloc, swt, axis=AX.X)
            stats2 = work.tile([1, 2], F32)
            nc.vector.tensor_copy(stats2[:, 0:1], mloc)
            nc.vector.tensor_copy(stats2[:, 1:2], sloc)

            # ---- AllGather stats -> global logsumexp ----
            ag3_in = dp.tile([1, 2], F32)
            ag3_out = dp.tile([2 * NCORES], F32, name="ag3_out", addr_space="Shared")
            nc.sync.dma_start(ag3_in, stats2)
            nc.gpsimd.collective_compute(
                "AllGather", mybir.AluOpType.bypass, replica_groups=rg,
                ins=[ag3_in.opt()], outs=[ag3_out.opt()],
            )
            gst = work.tile([1, 2 * NCORES], F32)
            nc.sync.dma_start(gst, ag3_out[None, :])
            g3 = gst.rearrange("p (k t) -> p k t", t=2)
            gm = work.tile([1, 1], F32)
            nc.vector.reduce_max(gm, g3[:, :, 0:1], axis=AX.XY)
            neggm = work.tile([1, 1], F32)
            nc.vector.tensor_scalar_mul(neggm, gm, -1.0)
            ew = work.tile([1, NCORES, 1], F32)
            nc.scalar.activation(ew, g3[:, :, 0:1], AF.Exp, bias=neggm, scale=1.0)
            sw2 = work.tile([1, NCORES, 1], F32)
            nc.vector.tensor_mul(sw2, ew, g3[:, :, 1:2])
            gs = work.tile([1, 1], F32)
            nc.vector.reduce_sum(gs, sw2, axis=AX.XY)
            lns = work.tile([1, 1], F32)
            nc.scalar.activation(lns, gs, AF.Ln)
            denom = work.tile([1, 1], F32)
            nc.vector.tensor_add(denom, gm, lns)
            negden = work.tile([1, 1], F32)
            nc.vector.tensor_scalar_mul(negden, denom, -1.0)
            bc_ps = pp.tile([128, 1], F32, tag="small", bufs=2, name="bc_ps")
            nc.tensor.matmul(bc_ps, ones_s, negden, start=True, stop=True)
            negden_b = work.tile([128, 1], F32)
            nc.vector.tensor_copy(negden_b, bc_ps)
            logp = work.tile([128, NV], F32)
            nc.vector.tensor_scalar_add(logp, logits, negden_b)
            nc.sync.dma_start(out_logp[:], logp)

    nc.compile()
    return nc


def _get_nc():
    if "nc" not in _CACHE:
        _CACHE["nc"] = _build()
    return _CACHE["nc"]


def _swiz_vec(v):
    """[1024] -> [128, 8] with [p, k] = v[k*128 + p]."""
    return np.ascontiguousarray(v.reshape(KH, 128).T)


def _shard_inputs(input_token, h0, c0, encoder_outputs, embedding, W_ih, W_hh,
                  b_ih, b_hh, W_ctx, b_ctx, W_out, b_out):
    f32 = np.float32
    tok = int(np.asarray(input_token).reshape(-1)[0])
    x = np.asarray(embedding[tok], dtype=f32)
    x_sw = _swiz_vec(x).astype(f32)
    h0_sw = _swiz_vec(np.asarray(h0, f32).reshape(H)).astype(f32)
    c0v = np.asarray(c0, f32).reshape(H)
    enc = np.asarray(encoder_outputs, f32)
    enc16 = enc.astype(NPBF16)
    encT16 = np.ascontiguousarray(
        enc.T.reshape(KH, 128, L).transpose(1, 0, 2).reshape(128, KH * L)
    ).astype(NPBF16)

    W_ih = np.asarray(W_ih, f32)
    W_hh = np.asarray(W_hh, f32)
    W_ctx = np.asarray(W_ctx, f32)
    W_out = np.asarray(W_out, f32)
    b_ih = np.asarray(b_ih, f32)
    b_hh = np.asarray(b_hh, f32)
    b_ctx = np.asarray(b_ctx, f32)
    b_out = np.asarray(b_out, f32)

    # replicated W_ctx tiles: [p, (m*16+k)*128 + j] = W_ctx[m*128 + j, k*128 + p]
    wctx16 = np.ascontiguousarray(
        W_ctx.reshape(KH, 128, K2H, 128).transpose(3, 0, 2, 1).reshape(128, KH * K2H * 128)
    ).astype(NPBF16)
    bctx_sw = np.ascontiguousarray(b_ctx.reshape(KH, 128).T).astype(f32)

    # vocab-padded W_out / b_out
    Wp = np.zeros((VP, H), f32)
    Wp[:V] = W_out
    bp = np.full(VP, PAD_BIAS, f32)
    bp[:V] = b_out

    ident = np.eye(128, dtype=f32)
    ones1 = np.ones((1, 128), f32)

    in_maps = []
    for c in range(NCORES):
        rows = (np.arange(4)[:, None] * H + c * 128 + np.arange(128)[None, :]).reshape(-1)
        Wsh = np.concatenate([W_ih[rows], W_hh[rows]], axis=1)  # [512, 2048]
        wcat16 = np.ascontiguousarray(
            Wsh.reshape(4, 128, K2H, 128).transpose(3, 0, 2, 1).reshape(128, 4 * K2H * 128)
        ).astype(NPBF16)
        sh = Wp[c * VS:(c + 1) * VS]
        wout16 = np.ascontiguousarray(
            sh.reshape(NV, 128, KH, 128).transpose(3, 0, 2, 1).reshape(128, NV * KH * 128)
        ).astype(NPBF16)
        bih_sw = np.ascontiguousarray(b_ih.reshape(4, KH, 128)[:, c, :].T).astype(f32)
        bhh_sw = np.ascontiguousarray(b_hh.reshape(4, KH, 128)[:, c, :].T).astype(f32)
        bout_sw = np.ascontiguousarray(bp[c * VS:(c + 1) * VS].reshape(NV, 128).T).astype(f32)
        in_maps.append({
            "x_sw": x_sw, "h0_sw": h0_sw,
            "c0_ch": np.ascontiguousarray(c0v[c * 128:(c + 1) * 128].reshape(128, 1)),
            "enc16": enc16, "encT16": encT16,
            "wcat16": wcat16, "wctx16": wctx16, "wout16": wout16,
            "bih_sw": bih_sw, "bhh_sw": bhh_sw, "bctx_sw": bctx_sw,
            "bout_sw": bout_sw, "ident": ident, "ones1": ones1,
        })
    return in_maps


def _unshard(results):
    logp = np.empty(VP, np.float32)
    h_new = np.empty(H, np.float32)
    c_new = np.empty(H, np.float32)
    for c in range(NCORES):
        r = results[c]
        logp[c * VS:(c + 1) * VS] = np.asarray(r["out_logp"]).T.reshape(-1)
        h_new[c * 128:(c + 1) * 128] = np.asarray(r["out_h"]).reshape(-1)
        c_new[c * 128:(c + 1) * 128] = np.asarray(r["out_c"]).reshape(-1)
    attn = np.asarray(results[0]["out_attn"]).reshape(L)
    return (
        logp[:V][None, :],
        h_new[None, None, :],
        c_new[None, None, :],
        attn,
    )


def kernel(**inputs):
    global LAST_RESULTS
    nc = _get_nc()
    in_maps = _shard_inputs(**inputs)
    res = run_bass_kernel_spmd(nc, in_maps, core_ids=list(range(NCORES)))
    LAST_RESULTS = res
    return _unshard(res.results)
